# revision 18
# baseline (speedup 1.0000x reference)
"""MOLELinear (mixture-of-linear-experts) Trainium2 kernel.

Math (per group g): out_g = x_g @ (sum_e c[g,e] W_e + W_sh).T + (sum_e c[g,e] b_e + b_sh)

Sharding: data-parallel over the 32 groups -> 4 groups (8192 tokens) per core,
expert weights replicated. Host does layout-only prep (transposes / stacking /
an identity constant / lossless bf16->f32 expansion of the output); all
arithmetic runs on device.

Device plan per core (DMA-roofline bound: 16 MiB x + 9 MiB w reads f32,
8 MiB out writes bf16 ~= 97 us at the 358 GB/s per-core HBM limit):
  - w then x stream on the SWDGE queue (f32 HBM -> bf16 SBUF cast inline in
    the DMA datapath, no engine ALU time); smalls ride the SP HWDGE ring in
    parallel. x chunks are big early (2048 tok) for stream efficiency and
    tapered (1024/512/512) for the last group to shorten the serial tail.
  - Weight mixing is split across two engines running concurrently so it
    hides entirely under the w/x DMA stream:
      PE (groups in PE_MIX): psum += (c[g,e]*I).T @ W_e chunks, shared weight
        added via a unit-identity matmul, plain ScalarE copy drains; the
        c[g,e]*I diagonals are built on DVE from a staged identity.
      DVE (groups in DVE_MIX): fused bf16 scalar_tensor_tensor chains
        (STT runs 1 elem/cycle regardless of dtype -- only 2 groups fit).
    Both produce wmixT[g] bf16 with k on partitions, (kt_local, o) free.
  - Main GEMM bf16: psum[o128, tok512] = sum_kt wmixT_slice.T @ xT_slice.
    Output is [o, tok] so the mixed bias is per-PARTITION.
  - Drain on ScalarE: activation(Identity, bias=bmix[:,col]) psum -> bf16 oc.
  - Per-chunk out DMAs on the ACT HWDGE ring (never head-of-line-blocks the
    input stream); host transposes + exactly expands bf16 -> f32.
"""
import ml_dtypes
import numpy as np

import concourse.bacc as bacc
import concourse.mybir as mybir
from concourse.alu_op_type import AluOpType
from concourse.tile import TileContext
from concourse.bass_utils import run_bass_kernel_spmd

N_CORES = 8
IN_F = 512
OUT_F = 512
N_EXPERTS = 8
N_GROUPS = 32
TOK_PER_GROUP = 2048
G_PER_CORE = N_GROUPS // N_CORES          # 4
TOK_PER_CORE = G_PER_CORE * TOK_PER_GROUP  # 8192
KT = IN_F // 128                           # 4 k-tiles
NE1 = N_EXPERTS + 1                        # experts + shared
F32 = mybir.dt.float32
F32R = mybir.dt.float32r
BF16 = mybir.dt.bfloat16

# x streaming schedule: (group, token offset, tokens) — big chunks early for
# stream efficiency, small chunks at the very end to shorten the tail
def _chunk_plan(gorder):
    plan = []
    for i, g in enumerate(gorder):
        if i < len(gorder) - 1:
            plan.append((g, 0, 2048))
        else:
            plan += [(g, 0, 1024), (g, 1024, 512), (g, 1536, 512)]
    return plan
HALF_COLS = NE1 * 2 * OUT_F                # 9216 cols per weight half

PE_MIX = [2, 3]                            # groups mixed on the PE
DVE_MIX = [0, 1]                           # groups mixed on the DVE
GORDER = [2, 3, 0, 1]                      # GEMM group order (mix readiness)

_CACHE = {}


def _build():
    nc = bacc.Bacc(trn_type="TRN2")
    xT = nc.dram_tensor("xT", (IN_F, TOK_PER_CORE), F32, kind="ExternalInput")
    wt = nc.dram_tensor("wt", (2, 128, HALF_COLS), F32, kind="ExternalInput")
    cb = nc.dram_tensor("cb", (128, G_PER_CORE * N_EXPERTS), F32, kind="ExternalInput")
    cx = nc.dram_tensor("cx", (NE1, G_PER_CORE), F32R, kind="ExternalInput")
    ball = nc.dram_tensor("ball", (NE1, OUT_F), F32R, kind="ExternalInput")
    ident = nc.dram_tensor("ident", (128, 128), BF16, kind="ExternalInput")
    outT = nc.dram_tensor("outT", (OUT_F, TOK_PER_CORE), BF16, kind="ExternalOutput")

    with TileContext(nc) as tc:
        with (
            tc.tile_pool(name="wp", bufs=1) as wp,
            tc.tile_pool(name="mixp", bufs=1) as mixp,
            tc.tile_pool(name="smallp", bufs=1) as smallp,
            tc.tile_pool(name="xp", bufs=3) as xp,
            tc.tile_pool(name="op", bufs=3) as op,
            tc.tile_pool(name="psp", bufs=4, space="PSUM") as psp,
            tc.tile_pool(name="psm", bufs=3, space="PSUM") as psm,
        ):
            # ---- smalls on the SP HWDGE ring (parallel to the SWDGE stream) ----
            cbt = smallp.tile([128, G_PER_CORE * N_EXPERTS], F32, tag="cb")
            nc.sync.dma_start(cbt[:], cb[:])
            identt = smallp.tile([128, 128], BF16, tag="ident")
            nc.sync.dma_start(identt[:], ident[:])
            cxt = smallp.tile([NE1, G_PER_CORE], F32R, tag="cx")
            nc.sync.dma_start(cxt[:], cx[:])
            ballt = smallp.tile([NE1, OUT_F], F32R, tag="ball")
            nc.sync.dma_start(ballt[:], ball[:])

            # ---- weight halves: SWDGE cast-DMA f32 -> bf16 ----
            wt_ap = wt[:]
            wall = []
            for h in range(2):
                t = wp.tile([128, HALF_COLS], BF16, tag=f"wh{h}")
                nc.gpsimd.dma_start(t[:], wt_ap[h])
                wall.append(t)

            # ---- ci[g,e] = c[g,e] * I on DVE (PE-mixed groups only) ----
            cit = smallp.tile([128, len(PE_MIX) * N_EXPERTS * 128], BF16, tag="ci")
            ci_col = {}
            for j, g in enumerate(PE_MIX):
                for e in range(N_EXPERTS):
                    i = j * N_EXPERTS + e
                    ci_col[(g, e)] = i
                    nc.vector.tensor_scalar_mul(
                        cit[:, i * 128 : (i + 1) * 128], identt[:],
                        cbt[:, g * N_EXPERTS + e : g * N_EXPERTS + e + 1],
                    )

            # ---- mixed biases on PE: pb[o128, g] = ballT_slice.T @ cxt ----
            # col layout of bmix: ot*G_PER_CORE + g
            pb = psm.tile([128, 4 * G_PER_CORE], F32, tag="pb", bufs=1)
            for ot in range(4):
                nc.tensor.matmul(
                    pb[:, ot * G_PER_CORE : (ot + 1) * G_PER_CORE],
                    ballt[:, ot * 128 : (ot + 1) * 128],
                    cxt[:],
                    start=True,
                    stop=True,
                )
            bmix = smallp.tile([128, 4 * G_PER_CORE], F32, tag="bmix")
            nc.scalar.copy(bmix[:], pb[:])

            # wm[g,h][p, kl*512+o] = sum_e c[g,e]*W_e^T[(2h+kl)*128+p, o] + sh
            wmix = {}
            for g in range(G_PER_CORE):
                for h in range(2):
                    wmix[(g, h)] = mixp.tile(
                        [128, 2 * OUT_F], BF16, tag=f"wm{g}_{h}", name=f"wm{g}_{h}"
                    )

            def emit_mix_pe(g, h):
                # psum += ci[g,e].T @ W_e chunks; shared via unit identity
                w = wall[h]
                for ch in range(2):
                    pm = psm.tile([128, 512], F32, tag="pm", bufs=2)
                    for e in range(N_EXPERTS):
                        nc.tensor.matmul(
                            pm[:],
                            cit[:, ci_col[(g, e)] * 128 : (ci_col[(g, e)] + 1) * 128],
                            w[:, e * 2 * OUT_F + ch * 512 : e * 2 * OUT_F + ch * 512 + 512],
                            start=(e == 0),
                            stop=False,
                        )
                    nc.tensor.matmul(
                        pm[:],
                        identt[:],
                        w[:, N_EXPERTS * 2 * OUT_F + ch * 512 : N_EXPERTS * 2 * OUT_F + ch * 512 + 512],
                        start=False,
                        stop=True,
                    )
                    nc.scalar.copy(wmix[(g, h)][:, ch * 512 : ch * 512 + 512], pm[:])

            def emit_mix_dve(g, h, acc):
                # fused bf16 STT chain: acc = c0*W0 + Wsh; acc = ce*We + acc
                w = wall[h]
                sh = w[:, N_EXPERTS * 2 * OUT_F : NE1 * 2 * OUT_F]
                nc.vector.scalar_tensor_tensor(
                    acc[:], w[:, 0 : 2 * OUT_F],
                    cbt[:, g * N_EXPERTS : g * N_EXPERTS + 1],
                    sh, AluOpType.mult, AluOpType.add,
                )
                for e in range(1, N_EXPERTS):
                    nc.vector.scalar_tensor_tensor(
                        acc[:] if e < N_EXPERTS - 1 else wmix[(g, h)][:],
                        w[:, e * 2 * OUT_F : (e + 1) * 2 * OUT_F],
                        cbt[:, g * N_EXPERTS + e : g * N_EXPERTS + e + 1],
                        acc[:], AluOpType.mult, AluOpType.add,
                    )

            # h0 halves for both engines, then h1 halves (w arrival order)
            acc = mixp.tile([128, 2 * OUT_F], BF16, tag="acc")
            for h in range(2):
                for g in PE_MIX:
                    emit_mix_pe(g, h)
                for g in DVE_MIX:
                    emit_mix_dve(g, h, acc)

            # ---- main GEMM in GORDER ----
            outT_ap = outT[:].rearrange("(ot p) t -> p ot t", p=128)
            for g, toff, ctok in _chunk_plan(GORDER):
                t0 = g * TOK_PER_GROUP + toff
                xs = xp.tile([128, KT * 2048], BF16, tag="x")
                nc.gpsimd.dma_start(
                    xs[:, : KT * ctok].rearrange("p (kt t) -> p kt t", kt=KT),
                    xT[:, t0 : t0 + ctok].rearrange("(kt p) t -> p kt t", p=128),
                )
                oc = op.tile([128, 4 * 2048], BF16, tag="oc")
                for sub in range(ctok // 512):
                    for ot in range(4):
                        ps = psp.tile([128, 512], F32, tag="ps")
                        for kt in range(KT):
                            h, kl = kt // 2, kt % 2
                            nc.tensor.matmul(
                                ps[:],
                                wmix[(g, h)][:, kl * OUT_F + ot * 128 : kl * OUT_F + ot * 128 + 128],
                                xs[:, kt * ctok + sub * 512 : kt * ctok + sub * 512 + 512],
                                start=(kt == 0),
                                stop=(kt == KT - 1),
                            )
                        # drain + per-partition bias add -> bf16
                        nc.scalar.activation(
                            oc[:, ot * ctok + sub * 512 : ot * ctok + sub * 512 + 512],
                            ps[:],
                            mybir.ActivationFunctionType.Identity,
                            bias=bmix[:, ot * G_PER_CORE + g : ot * G_PER_CORE + g + 1],
                            scale=1.0,
                        )
                # per-chunk out DMA on the ACT ring
                nc.scalar.dma_start(
                    outT_ap[:, :, t0 : t0 + ctok],
                    oc[:, : 4 * ctok].rearrange("p (ot t) -> p ot t", ot=4),
                )
    nc.finalize()
    return nc


def kernel(x, coefficients, weight_experts, bias_experts, weight_shared, bias_shared, sizes):
    x = np.asarray(x)
    coefficients = np.asarray(coefficients)
    weight_experts = np.asarray(weight_experts)
    bias_experts = np.asarray(bias_experts)
    weight_shared = np.asarray(weight_shared)
    bias_shared = np.asarray(bias_shared)

    if "nc" not in _CACHE:
        _CACHE["nc"] = _build()
    nc = _CACHE["nc"]

    # ---- host-side layout prep (no arithmetic) ----
    # wt[h, p, e, kl, o] = W_e[o, (2h+kl)*128 + p]  (e==8 -> shared)
    WT = np.concatenate([weight_experts, weight_shared[None]], axis=0).transpose(0, 2, 1)
    wt_np = np.ascontiguousarray(
        WT.reshape(NE1, 2, 2, 128, OUT_F).transpose(1, 3, 0, 2, 4)
    ).reshape(2, 128, HALF_COLS)
    ball_np = np.empty((NE1, OUT_F), np.float32)
    ball_np[:N_EXPERTS] = bias_experts
    ball_np[N_EXPERTS] = bias_shared
    ident_np = np.eye(128, dtype=ml_dtypes.bfloat16)

    in_maps = []
    for c in range(N_CORES):
        gs = slice(c * G_PER_CORE, (c + 1) * G_PER_CORE)
        cg = coefficients[gs]  # [4, 8]
        cb_np = np.broadcast_to(
            cg.reshape(1, -1), (128, G_PER_CORE * N_EXPERTS)
        ).copy()
        cx_np = np.empty((NE1, G_PER_CORE), np.float32)
        cx_np[:N_EXPERTS] = cg.T
        cx_np[N_EXPERTS] = 1.0
        xT_np = np.ascontiguousarray(
            x[c * TOK_PER_CORE : (c + 1) * TOK_PER_CORE].T
        )
        in_maps.append(
            {
                "xT": xT_np,
                "wt": wt_np,
                "cb": cb_np,
                "cx": cx_np,
                "ball": ball_np,
                "ident": ident_np,
            }
        )

    res = run_bass_kernel_spmd(nc, in_maps, core_ids=list(range(N_CORES)))
    # outT [512, 8192] bf16 per core -> [8192, 512] f32 (exact expansion)
    return np.concatenate(
        [res.results[c]["outT"].T.astype(np.float32) for c in range(N_CORES)], axis=0
    )
